# revision 37
# baseline (speedup 1.0000x reference)
"""Contrastive loss (NT-Xent style) Trainium2 kernel, 8-core SPMD, fp8.

Math: with z_i = normalize(instance_emb.reshape(4096, 512)),
zbag = normalize(bag_emb) [8, 512], Z = [z_i; repeat(zbag, 512)], the
reference loss reduces to (see _assemble):

  denom_i[r] = rowsum(exp(2 G[r,:])) - e^2 + 512 * sum_g exp(2 S1[r,g])
  denom_j[g] = colsum_r(exp(2 S1[r,g])) + 512 * rowsum(exp(2 B[g,:])) - e^2
  loss*8192 = sum_r [log denom_i[r] - 4*S1[r, r//512]] + 512*sum_g log denom_j[g]

with G = z_i z_i^T, S1 = z_i zbag^T, B = zbag zbag^T.

No collectives (a 2KB AllGather costs ~70us fixed in this rig): the
host replicates fp8(16*Y^T) to every core, cyclically rotated so each
core's own 512 rows are local block 0.  Per core:

- G block [own 512 x 4096] in fp8 DoubleRow matmuls (K=256/instr) on
  raw y16 values; normalization applied late per column-pair: DVE
  multiplies the PSUM block by a broadcast rb_c = 1/(16||y||) tile,
  then exp runs with per-partition scale 2*rb_own (both norms exact).
- sumsq for the norms comes from the transposed fp8 tiles: squares
  (bf16 out) spread across scalar/vector/gpsimd + PE ones-matmul
  partition-reduction -> [8,512] PSUM tiles, bounced through DRAM into
  a [128, 32] per-partition layout; the rb broadcast tiles are built
  with K=1 ones-matmuls (a [128,1024] f32 broadcast DMA costs ~35us
  on one queue here -- never DMA-broadcast wide tiles).
- rsqrt = linear seed around E||16y||^2 = 131072 + 1 Newton step,
  vector-only, so the scalar engine keeps its Exp table loaded
  (ACT table reloads cost 1.5us each).
Output: per-core scalar partials; host sums O(8) values.
"""

import os
import numpy as np
import ml_dtypes
from contextlib import ExitStack

import concourse.bass as bass
import concourse.bacc as bacc
import concourse.tile as tile
from concourse import mybir
from concourse import bass_utils
from concourse.masks import make_identity

F32 = mybir.dt.float32
BF16 = mybir.dt.bfloat16
F8 = mybir.dt.float8e4
DR = mybir.MatmulPerfMode.DoubleRow

B, N, D = 8, 512, 512
BS = B * N
NCORES = 8
RPC = BS // NCORES      # 512 rows per core
TPC = RPC // 128        # 4 row-tiles per core
NBLK = BS // 512        # 8 column blocks
NPAIR = NBLK // 2       # 4 column block-pairs
E2 = float(np.exp(2.0))

# linear rsqrt seed around ss8 = ||16 y||^2 ~= 256*512
SS0 = 256.0 * 512.0
R0 = SS0 ** -0.5
LIN_A = 1.5 * R0
LIN_B = R0 / (2.0 * SS0)

LAST_EXEC_TIME_NS = None
_CACHED_NC = None


def _newton(nc, work, r, ss_slice, width, tag, iters=1):
    a = work.tile([128, width], F32, name=f"nta_{tag}")
    for _ in range(iters):
        nc.vector.tensor_mul(a, r, r)
        nc.vector.tensor_mul(a, a, ss_slice)
        nc.vector.tensor_scalar(
            out=a, in0=a, scalar1=-0.5, scalar2=1.5,
            op0=mybir.AluOpType.mult, op1=mybir.AluOpType.add,
        )
        nc.vector.tensor_mul(r, r, a)


def _build_kernel(nc):
    # fp8(16*Y^T), columns in local cyclic order
    ytb = nc.dram_tensor("ytb", [D, BS], F8, kind="ExternalInput")
    bag = nc.dram_tensor("bag", [B, D], F32, kind="ExternalInput")
    onehot = nc.dram_tensor("onehot", [1, B], F32, kind="ExternalInput")
    out_d = nc.dram_tensor("out", [1, 18], F32, kind="ExternalOutput")

    with ExitStack() as ctx:
        tc = ctx.enter_context(tile.TileContext(nc))

        consts = ctx.enter_context(tc.tile_pool(name="consts", bufs=1))
        work = ctx.enter_context(tc.tile_pool(name="work", bufs=2))
        persist = ctx.enter_context(tc.tile_pool(name="persist", bufs=1))
        dram = ctx.enter_context(tc.tile_pool(name="dram", bufs=1, space="DRAM"))
        ps_main = ctx.enter_context(tc.tile_pool(name="ps_main", bufs=2, space="PSUM"))
        expool = ctx.enter_context(tc.tile_pool(name="expool", bufs=3))
        ps_sm = ctx.enter_context(tc.tile_pool(name="ps_sm", bufs=4, space="PSUM"))

        ident = consts.tile([128, 128], F32)
        make_identity(nc, ident)
        ones = consts.tile([128, 1], F32)
        nc.vector.memset(ones, 1.0)
        ones8 = consts.tile([128, 8], BF16)
        nc.vector.memset(ones8, 1.0)
        ones1 = consts.tile([1, 128], F32)
        nc.vector.memset(ones1, 1.0)
        oh = consts.tile([128, B], F32)
        nc.sync.dma_start(out=oh, in_=onehot.ap().to_broadcast((128, B)))
        bag_t = persist.tile([B, D], F32, name="bag_t")
        nc.sync.dma_start(out=bag_t, in_=bag[:, :])

        # ---- input DMA: yt8[kk] = [128(p), 2(j), 4096(c)], d = 256kk+128j+p
        yt8 = [persist.tile([128, 2, BS], F8, name=f"yt8_{kk}") for kk in range(2)]

        def load_piece(cq, kk, j):
            c0 = cq * 1024
            if cq == 0:
                for rh in range(2):
                    nc.sync.dma_start(
                        out=yt8[kk][rh * 64:(rh + 1) * 64, j, c0:c0 + 1024],
                        in_=ytb[256 * kk + 128 * j + 64 * rh:
                                256 * kk + 128 * j + 64 * (rh + 1),
                                c0:c0 + 1024],
                    )
            else:
                nc.sync.dma_start(
                    out=yt8[kk][:, j, c0:c0 + 1024],
                    in_=ytb[256 * kk + 128 * j:256 * kk + 128 * (j + 1),
                            c0:c0 + 1024],
                )

        # order: cq0 (everything starts there), then cq3-kk1 (gpsimd
        # squares gate the pair-3 chain and gpsimd is otherwise idle),
        # then the rest in consumption order
        for kk in range(2):
            for j in range(2):
                load_piece(0, kk, j)
        for j in range(2):
            load_piece(3, 1, j)
        for cq in (1, 2):
            for kk in range(2):
                for j in range(2):
                    load_piece(cq, kk, j)
        for j in range(2):
            load_piece(3, 0, j)

        # ---- bag chain (fp32, tiny): normalize, scale by 16, to fp8 ----
        sqb = work.tile([B, D], F32, name="sq_bag")
        nc.vector.tensor_mul(sqb, bag_t, bag_t)
        ssb = persist.tile([B, 1], F32, name="ss_bag")
        nc.vector.reduce_sum(ssb, sqb, axis=mybir.AxisListType.X)
        rbg = work.tile([B, 1], F32, name="r_bag")
        # seed for ss ~ 512 (bag rows are unscaled)
        nc.vector.tensor_scalar(
            out=rbg, in0=ssb, scalar1=-(512.0 ** -0.5) / 1024.0,
            scalar2=1.5 * 512.0 ** -0.5,
            op0=mybir.AluOpType.mult, op1=mybir.AluOpType.add,
        )
        a = work.tile([B, 1], F32, name="nta_bag")
        for _ in range(3):
            nc.vector.tensor_mul(a, rbg, rbg)
            nc.vector.tensor_mul(a, a, ssb)
            nc.vector.tensor_scalar(
                out=a, in0=a, scalar1=-0.5, scalar2=1.5,
                op0=mybir.AluOpType.mult, op1=mybir.AluOpType.add,
            )
            nc.vector.tensor_mul(rbg, rbg, a)
        nc.vector.tensor_scalar(
            out=rbg, in0=rbg, scalar1=16.0, scalar2=None, op0=mybir.AluOpType.mult
        )
        zbag16 = persist.tile([B, D], F32, name="zbag16")
        nc.vector.tensor_scalar_mul(zbag16, bag_t, rbg[:, 0:1])
        # zbagT8[kk] = [128, 2, 16] fp8 (B padded to 16 for the DoubleRow
        # 16-byte Ko-stride ISA rule; pad columns are zero)
        zbagT8 = [persist.tile([128, 2, 16], F8, name=f"zbagT8_{kk}") for kk in range(2)]
        for kk in range(2):
            nc.vector.memset(zbagT8[kk], 0.0)
            for j in range(2):
                ptr = ps_sm.tile([128, B], F32, tag="sm", name="ptr_bag")
                d0 = 256 * kk + 128 * j
                nc.tensor.transpose(ptr, zbag16[:, d0:d0 + 128], ident[:B, :B])
                nc.vector.tensor_copy(zbagT8[kk][:, j, 0:B], ptr)

        # ---- S1 own rows (raw lhsT); stash raw psum in SBUF ----
        s1rs = persist.tile([128, TPC], F32, name="s1rs")
        pos = persist.tile([128, TPC], F32, name="pos")
        es1 = persist.tile([128, TPC, B], F32, name="es1")
        s1sc = persist.tile([128, B], F32, name="s1sc")
        s1raw = persist.tile([128, TPC, B], F32, name="s1raw")
        for t in range(TPC):
            pm = ps_sm.tile([128, 16], F32, tag="sm", name="ps_s1")
            for kk in range(2):
                nc.tensor.matmul(
                    pm,
                    lhsT=yt8[kk][:, :, t * 128:(t + 1) * 128],
                    rhs=zbagT8[kk],
                    start=(kk == 0), stop=(kk == 1),
                    perf_mode=DR,
                )
            nc.vector.tensor_copy(s1raw[:, t, :], pm[:, 0:B])

        # ---- sumsq via squares + PE ones-matmul partition reduce ----
        sq16 = [persist.tile([128, 2, BS], BF16, name=f"sq16_{kk}") for kk in range(2)]
        ssd = [dram.tile([1, 1024], F32, name=f"ssd_{j}") for j in range(NPAIR)]
        ssrow = [persist.tile([8, 1024], F32, name=f"ssrow_{j}") for j in range(NPAIR)]
        ss = persist.tile([128, NBLK * TPC], F32, name="ss")
        rb = persist.tile([128, NBLK * TPC], F32, name="rb")
        rbc = [persist.tile([128, 1024], BF16, name=f"rbc_{j}") for j in range(NPAIR)]
        rbd = [dram.tile([1, 1024], BF16, name=f"rbd_{j}") for j in range(NPAIR)]

        def sumsq_quarter(cq):
            """squares (spread across engines) + ones-matmul partition
            reduce into ssrow[cq], bounced to DRAM"""
            c0 = cq * 1024
            for kk in range(2):
                for j in range(2):
                    if cq == 0:
                        eng = nc.scalar if kk == 0 else nc.vector
                    elif cq == 1:
                        eng = nc.scalar
                    elif cq == 2:
                        eng = nc.vector
                    else:
                        eng = nc.scalar if kk == 0 else nc.gpsimd
                    if eng is nc.scalar:
                        eng.activation(
                            sq16[kk][:, j, c0:c0 + 1024],
                            yt8[kk][:, j, c0:c0 + 1024],
                            mybir.ActivationFunctionType.Square,
                        )
                    else:
                        eng.tensor_mul(
                            sq16[kk][:, j, c0:c0 + 1024],
                            yt8[kk][:, j, c0:c0 + 1024],
                            yt8[kk][:, j, c0:c0 + 1024],
                        )
            for h in range(2):
                cb = cq * 2 + h
                pss = ps_sm.tile([8, 512], F32, tag="sm", name=f"ps_ss{cb}")
                first = True
                for kk in range(2):
                    for j in range(2):
                        nc.tensor.matmul(
                            pss, lhsT=ones8,
                            rhs=sq16[kk][:, j, cb * 512:(cb + 1) * 512],
                            start=first, stop=(kk == 1 and j == 1),
                        )
                        first = False
                if cq < 2:
                    nc.vector.tensor_copy(ssrow[cq][:, h * 512:(h + 1) * 512], pss)
                else:
                    # keep the vector queue clear for the pair-0/1 chains:
                    # these copies would otherwise block them behind the
                    # slower gpsimd squares (in-order queue)
                    nc.scalar.copy(ssrow[cq][:, h * 512:(h + 1) * 512], pss)
            if cq < 2:
                nc.sync.dma_start(out=ssd[cq], in_=ssrow[cq][0:1, :])
            else:
                # sync is in-order: these wait on the scalar copies above and
                # would block the pair-0/1 ssback DMAs behind them
                nc.scalar.dma_start(out=ssd[cq], in_=ssrow[cq][0:1, :])

        def rsqrt_pair(pj, prescale_scale):
            """rb for pair pj from ssd[pj]; broadcast tile rbc[pj]
            (times 16 for prescale pairs: zt8 = y16*16*rb = 16*z)."""
            c0 = pj * 8
            nc.sync.dma_start(
                out=ss[:, c0:c0 + 8],
                in_=ssd[pj].rearrange("1 (m p) -> p m", p=128),
            )
            nc.vector.tensor_scalar(
                out=rb[:, c0:c0 + 8], in0=ss[:, c0:c0 + 8],
                scalar1=-LIN_B, scalar2=LIN_A,
                op0=mybir.AluOpType.mult, op1=mybir.AluOpType.add,
            )
            _newton(nc, work, rb[:, c0:c0 + 8], ss[:, c0:c0 + 8], 8, f"p{pj}")
            ptr = ps_sm.tile([8, 128], F32, tag="sm", name=f"ptr_rb{pj}")
            nc.tensor.transpose(ptr, rb[:, c0:c0 + 8], ident)
            rT = work.tile([8, 128], BF16, name=f"rT_{pj}")
            nc.vector.tensor_scalar(
                out=rT, in0=ptr, scalar1=prescale_scale, scalar2=None,
                op0=mybir.AluOpType.mult,
            )
            nc.sync.dma_start(
                out=rbd[pj].rearrange("1 (t p) -> t p", t=8), in_=rT
            )
            # 4 parallel 64KB bf16 broadcast DMAs (a single 512KB f32
            # broadcast runs ~35us on one queue; K=1 PE matmuls steal
            # ~0.7us each from the main-loop stream)
            for q in range(4):
                nc.sync.dma_start(
                    out=rbc[pj][q * 32:(q + 1) * 32, :],
                    in_=rbd[pj].to_broadcast((32, 1024)),
                )

        sumsq_quarter(0)
        sumsq_quarter(1)
        sumsq_quarter(2)
        sumsq_quarter(3)
        rsqrt_pair(0, 1.0)
        rsqrt_pair(1, 1.0)
        rsqrt_pair(2, 1.0)
        rsqrt_pair(3, 1.0)

        # scale APs: r2own = 2*rb_own (raw+fixup), rs1own = rb_own/8 (prescaled/S1)
        r2own = persist.tile([128, TPC], F32, name="r2own")
        nc.vector.tensor_scalar(
            out=r2own, in0=rb[:, 0:TPC], scalar1=2.0, scalar2=None,
            op0=mybir.AluOpType.mult,
        )
        rs1own = persist.tile([128, TPC], F32, name="rs1own")
        nc.vector.tensor_scalar(
            out=rs1own, in0=rb[:, 0:TPC], scalar1=0.125, scalar2=None,
            op0=mybir.AluOpType.mult,
        )

        # es1 + positives (s1raw = 256 * y.zbag)
        for t in range(TPC):
            nc.scalar.activation(
                es1[:, t, :], s1raw[:, t, :], mybir.ActivationFunctionType.Exp,
                scale=rs1own[:, t:t + 1], accum_out=s1rs[:, t:t + 1],
            )
            nc.vector.tensor_mul(s1sc, s1raw[:, t, :], oh)
            nc.vector.reduce_sum(pos[:, t:t + 1], s1sc, axis=mybir.AxisListType.X)
        nc.vector.tensor_mul(pos, pos, rb[:, 0:TPC])  # pos = 16*sim, folded later

        # ---- Bgram ----
        pbg = ps_sm.tile([B, B], F32, tag="sm", name="ps_bgram")
        for kk in range(2):
            for j in range(2):
                nc.tensor.matmul(
                    pbg, lhsT=zbagT8[kk][:, j, 0:B], rhs=zbagT8[kk][:, j, 0:B],
                    start=(kk == 0 and j == 0), stop=(kk == 1 and j == 1),
                )
        ebg = persist.tile([B, B], F32, name="exp_bgram")
        rsbg = persist.tile([B, 1], F32, name="rs_bgram")
        nc.scalar.activation(
            ebg, pbg, mybir.ActivationFunctionType.Exp, scale=2.0 / 256.0,
            accum_out=rsbg,
        )

        # colsum over own rows of exp(2 S1own) + rsbg row: off the tail
        pv = ps_sm.tile([1, B], F32, tag="sm", name="ps_v")
        for t in range(TPC):
            nc.tensor.matmul(
                pv, lhsT=ones, rhs=es1[:, t, :],
                start=(t == 0), stop=(t == TPC - 1),
            )
        vrow = persist.tile([1, B], F32, name="vrow")
        nc.vector.tensor_copy(vrow, pv)
        prb = ps_sm.tile([1, B], F32, tag="sm", name="ps_rbT")
        nc.tensor.transpose(prb, rsbg, ident[:B, :B])
        rsbgT = persist.tile([1, B], F32, name="rsbgT")
        nc.vector.tensor_copy(rsbgT, prb)

        # ---- main loop ----
        rs = persist.tile([128, TPC, NPAIR], F32, name="rs")
        for bb in range(NPAIR):
            for t in range(TPC):
                pm = ps_main.tile([128, 1024], F32, name="ps_g")
                for kk in range(2):
                    for half in range(2):
                        blk = 2 * bb + half
                        nc.tensor.matmul(
                            pm[:, half * 512:(half + 1) * 512],
                            lhsT=yt8[kk][:, :, t * 128:(t + 1) * 128],
                            rhs=yt8[kk][:, :, blk * 512:(blk + 1) * 512],
                            start=(kk == 0), stop=(kk == 1),
                            perf_mode=DR,
                        )
                # fixup writes bf16 to SBUF: the PSUM tile frees here (not
                # after the exp), shortening the PE's psum-recycle chain
                fx = expool.tile([128, 1024], BF16, name="fx")
                nc.vector.tensor_mul(fx, pm, rbc[bb])
                nc.scalar.activation(
                    fx, fx, mybir.ActivationFunctionType.Exp,
                    scale=r2own[:, t:t + 1], accum_out=rs[:, t, bb:bb + 1],
                )

        # ---- denominators + logs ----
        rsum = persist.tile([128, TPC], F32, name="rsum")
        nc.vector.reduce_sum(rsum, rs, axis=mybir.AxisListType.X)
        di = persist.tile([128, TPC], F32, name="di")
        nc.vector.tensor_scalar(
            out=di, in0=s1rs, scalar1=512.0, scalar2=-E2,
            op0=mybir.AluOpType.mult, op1=mybir.AluOpType.add,
        )
        nc.vector.tensor_add(di, di, rsum)
        ldi = persist.tile([128, TPC], F32, name="ldi")
        nc.scalar.activation(ldi, di, mybir.ActivationFunctionType.Ln)

        # fin = sum_t ldi - (4/16)*sum_t pos
        fin = persist.tile([128, 1], F32, name="fin")
        vsum = persist.tile([128, 1], F32, name="vsum")
        nc.vector.reduce_sum(vsum, ldi, axis=mybir.AxisListType.X)
        posr = persist.tile([128, 1], F32, name="posr")
        nc.vector.reduce_sum(posr, pos, axis=mybir.AxisListType.X)
        nc.vector.tensor_scalar(
            out=posr, in0=posr, scalar1=-0.25, scalar2=None,
            op0=mybir.AluOpType.mult,
        )
        nc.vector.tensor_add(fin, vsum, posr)

        pfin = ps_sm.tile([1, 1], F32, tag="sm", name="ps_fin")
        nc.tensor.matmul(pfin, lhsT=ones, rhs=fin, start=True, stop=True)
        outt = persist.tile([1, 18], F32, name="outt")
        nc.vector.memset(outt, 0.0)
        nc.vector.tensor_copy(outt[:, 0:1], pfin)
        nc.vector.tensor_copy(outt[:, 2:10], vrow)
        nc.vector.tensor_copy(outt[:, 10:18], rsbgT)
        nc.sync.dma_start(out=out_d[:, :], in_=outt)

    return nc


def _get_nc():
    global _CACHED_NC
    if _CACHED_NC is None:
        nc = bacc.Bacc(
            "TRN2", target_bir_lowering=False, debug=False, num_devices=NCORES
        )
        nc = _build_kernel(nc)
        nc.compile()
        _CACHED_NC = nc
    return _CACHED_NC


def kernel(instance_emb: np.ndarray, bag_emb: np.ndarray) -> np.ndarray:
    global LAST_EXEC_TIME_NS
    Y = np.asarray(instance_emb, dtype=np.float32).reshape(BS, D)
    bg = np.ascontiguousarray(np.asarray(bag_emb, dtype=np.float32))

    in_maps = []
    for c in range(NCORES):
        Yc = np.roll(Y, -c * RPC, axis=0)
        yt16 = np.clip(Yc.T * 16.0, -240.0, 240.0)
        ytb_c = np.ascontiguousarray(yt16.astype(ml_dtypes.float8_e4m3))
        ohv = np.zeros((1, B), np.float32)
        ohv[0, c] = 1.0
        in_maps.append({"ytb": ytb_c, "bag": bg, "onehot": ohv})

    nc = _get_nc()
    trace = os.environ.get("CL_KERNEL_TRACE", "0") == "1"
    tmpdir = os.environ.get("CL_KERNEL_TRACE_DIR") or None
    if os.environ.get("CL_KERNEL_WARMUP", "0") == "1":
        bass_utils.run_bass_kernel_spmd(
            nc, in_maps, core_ids=list(range(NCORES)), trace=False
        )
    res = bass_utils.run_bass_kernel_spmd(
        nc, in_maps, core_ids=list(range(NCORES)), trace=trace, tmpdir=tmpdir
    )
    LAST_EXEC_TIME_NS = res.exec_time_ns

    return _assemble([res.results[c]["out"] for c in range(NCORES)])


def _assemble(outs) -> np.ndarray:
    """out row = [partial_c, pad, v_c[0:8], rsbg[0:8]];
    denom_j[g] = sum_c v_c[g] + 512*rsbg[g] - e^2."""
    total = 0.0
    vsum = np.zeros(B, np.float64)
    for c in range(NCORES):
        o = np.asarray(outs[c], np.float64).reshape(-1)
        total += o[0]
        vsum += o[2:10]
    rsbg = np.asarray(outs[0], np.float64).reshape(-1)[10:18]
    denom_j = vsum + 512.0 * rsbg - E2
    lj = 512.0 * float(np.sum(np.log(denom_j)))
    return np.float32((total + lj) / (2 * BS))


# revision 38
# speedup vs baseline: 1.1117x; 1.1117x over previous
"""Contrastive loss (NT-Xent style) Trainium2 kernel, 8-core SPMD, fp8.

Math: with z_i = normalize(instance_emb.reshape(4096, 512)),
zbag = normalize(bag_emb) [8, 512], Z = [z_i; repeat(zbag, 512)], the
reference loss reduces to (see _assemble):

  denom_i[r] = rowsum(exp(2 G[r,:])) - e^2 + 512 * sum_g exp(2 S1[r,g])
  denom_j[g] = colsum_r(exp(2 S1[r,g])) + 512 * rowsum(exp(2 B[g,:])) - e^2
  loss*8192 = sum_r [log denom_i[r] - 4*S1[r, r//512]] + 512*sum_g log denom_j[g]

with G = z_i z_i^T, S1 = z_i zbag^T, B = zbag zbag^T.

No collectives (a 2KB AllGather costs ~70us fixed in this rig): the
host replicates fp8(16*Y^T) to every core, cyclically rotated so each
core's own 512 rows are local block 0.  Per core:

- G block [own 512 x 4096] in fp8 DoubleRow matmuls (K=256/instr) on
  raw y16 values; normalization applied late per column-pair: DVE
  multiplies the PSUM block by a broadcast rb_c = 1/(16||y||) tile,
  then exp runs with per-partition scale 2*rb_own (both norms exact).
- sumsq for the norms comes from the transposed fp8 tiles: squares
  (bf16 out) spread across scalar/vector/gpsimd + PE ones-matmul
  partition-reduction -> [8,512] PSUM tiles, bounced through DRAM into
  a [128, 32] per-partition layout; the rb broadcast tiles are built
  with K=1 ones-matmuls (a [128,1024] f32 broadcast DMA costs ~35us
  on one queue here -- never DMA-broadcast wide tiles).
- rsqrt = linear seed around E||16y||^2 = 131072 + 1 Newton step,
  vector-only, so the scalar engine keeps its Exp table loaded
  (ACT table reloads cost 1.5us each).
Output: per-core scalar partials; host sums O(8) values.
"""

import os
import numpy as np
import ml_dtypes
from contextlib import ExitStack

import concourse.bass as bass
import concourse.bacc as bacc
import concourse.tile as tile
from concourse import mybir
from concourse import bass_utils
from concourse.masks import make_identity

F32 = mybir.dt.float32
BF16 = mybir.dt.bfloat16
F8 = mybir.dt.float8e4
DR = mybir.MatmulPerfMode.DoubleRow

B, N, D = 8, 512, 512
BS = B * N
NCORES = 8
RPC = BS // NCORES      # 512 rows per core
TPC = RPC // 128        # 4 row-tiles per core
NBLK = BS // 512        # 8 column blocks
NPAIR = NBLK // 2       # 4 column block-pairs
E2 = float(np.exp(2.0))

# linear rsqrt seed around ss8 = ||16 y||^2 ~= 256*512
SS0 = 256.0 * 512.0
R0 = SS0 ** -0.5
LIN_A = 1.5 * R0
LIN_B = R0 / (2.0 * SS0)

LAST_EXEC_TIME_NS = None
_CACHED_NC = None


def _newton(nc, work, r, ss_slice, width, tag, iters=1):
    a = work.tile([128, width], F32, name=f"nta_{tag}")
    for _ in range(iters):
        nc.vector.tensor_mul(a, r, r)
        nc.vector.tensor_mul(a, a, ss_slice)
        nc.vector.tensor_scalar(
            out=a, in0=a, scalar1=-0.5, scalar2=1.5,
            op0=mybir.AluOpType.mult, op1=mybir.AluOpType.add,
        )
        nc.vector.tensor_mul(r, r, a)


def _build_kernel(nc):
    # fp8(16*Y^T), columns in local cyclic order
    ytb = nc.dram_tensor("ytb", [D, BS], F8, kind="ExternalInput")
    bag = nc.dram_tensor("bag", [B, D], F32, kind="ExternalInput")
    onehot = nc.dram_tensor("onehot", [1, B], F32, kind="ExternalInput")
    out_d = nc.dram_tensor("out", [1, 18], F32, kind="ExternalOutput")

    with ExitStack() as ctx:
        tc = ctx.enter_context(tile.TileContext(nc))

        consts = ctx.enter_context(tc.tile_pool(name="consts", bufs=1))
        work = ctx.enter_context(tc.tile_pool(name="work", bufs=2))
        persist = ctx.enter_context(tc.tile_pool(name="persist", bufs=1))
        dram = ctx.enter_context(tc.tile_pool(name="dram", bufs=1, space="DRAM"))
        ps_main = ctx.enter_context(tc.tile_pool(name="ps_main", bufs=3, space="PSUM"))
        expool = ctx.enter_context(tc.tile_pool(name="expool", bufs=3))
        ps_sm = ctx.enter_context(tc.tile_pool(name="ps_sm", bufs=2, space="PSUM"))

        ident = consts.tile([128, 128], F32)
        make_identity(nc, ident)
        ones = consts.tile([128, 1], F32)
        nc.vector.memset(ones, 1.0)
        ones8 = consts.tile([128, 8], BF16)
        nc.vector.memset(ones8, 1.0)
        ones1 = consts.tile([1, 128], F32)
        nc.vector.memset(ones1, 1.0)
        oh = consts.tile([128, B], F32)
        nc.sync.dma_start(out=oh, in_=onehot.ap().to_broadcast((128, B)))
        bag_t = persist.tile([B, D], F32, name="bag_t")
        nc.sync.dma_start(out=bag_t, in_=bag[:, :])

        # ---- input DMA: yt8[kk] = [128(p), 2(j), 4096(c)], d = 256kk+128j+p
        yt8 = [persist.tile([128, 2, BS], F8, name=f"yt8_{kk}") for kk in range(2)]

        def load_piece(cq, kk, j):
            c0 = cq * 1024
            if cq == 0:
                for rh in range(2):
                    nc.sync.dma_start(
                        out=yt8[kk][rh * 64:(rh + 1) * 64, j, c0:c0 + 1024],
                        in_=ytb[256 * kk + 128 * j + 64 * rh:
                                256 * kk + 128 * j + 64 * (rh + 1),
                                c0:c0 + 1024],
                    )
            else:
                nc.sync.dma_start(
                    out=yt8[kk][:, j, c0:c0 + 1024],
                    in_=ytb[256 * kk + 128 * j:256 * kk + 128 * (j + 1),
                            c0:c0 + 1024],
                )

        # order: cq0 (everything starts there), then cq3-kk1 (gpsimd
        # squares gate the pair-3 chain and gpsimd is otherwise idle),
        # then the rest in consumption order
        for kk in range(2):
            for j in range(2):
                load_piece(0, kk, j)
        for j in range(2):
            load_piece(3, 1, j)
        for cq in (1, 2):
            for kk in range(2):
                for j in range(2):
                    load_piece(cq, kk, j)
        for j in range(2):
            load_piece(3, 0, j)

        # ---- bag chain (fp32, tiny): normalize, scale by 16, to fp8 ----
        sqb = work.tile([B, D], F32, name="sq_bag")
        nc.vector.tensor_mul(sqb, bag_t, bag_t)
        ssb = persist.tile([B, 1], F32, name="ss_bag")
        nc.vector.reduce_sum(ssb, sqb, axis=mybir.AxisListType.X)
        rbg = work.tile([B, 1], F32, name="r_bag")
        # seed for ss ~ 512 (bag rows are unscaled)
        nc.vector.tensor_scalar(
            out=rbg, in0=ssb, scalar1=-(512.0 ** -0.5) / 1024.0,
            scalar2=1.5 * 512.0 ** -0.5,
            op0=mybir.AluOpType.mult, op1=mybir.AluOpType.add,
        )
        a = work.tile([B, 1], F32, name="nta_bag")
        for _ in range(3):
            nc.vector.tensor_mul(a, rbg, rbg)
            nc.vector.tensor_mul(a, a, ssb)
            nc.vector.tensor_scalar(
                out=a, in0=a, scalar1=-0.5, scalar2=1.5,
                op0=mybir.AluOpType.mult, op1=mybir.AluOpType.add,
            )
            nc.vector.tensor_mul(rbg, rbg, a)
        nc.vector.tensor_scalar(
            out=rbg, in0=rbg, scalar1=16.0, scalar2=None, op0=mybir.AluOpType.mult
        )
        zbag16 = persist.tile([B, D], F32, name="zbag16")
        nc.vector.tensor_scalar_mul(zbag16, bag_t, rbg[:, 0:1])
        # zbagT8[kk] = [128, 2, 16] fp8 (B padded to 16 for the DoubleRow
        # 16-byte Ko-stride ISA rule; pad columns are zero)
        zbagT8 = [persist.tile([128, 2, 16], F8, name=f"zbagT8_{kk}") for kk in range(2)]
        for kk in range(2):
            nc.vector.memset(zbagT8[kk], 0.0)
            for j in range(2):
                ptr = ps_sm.tile([128, B], F32, tag="sm", name="ptr_bag")
                d0 = 256 * kk + 128 * j
                nc.tensor.transpose(ptr, zbag16[:, d0:d0 + 128], ident[:B, :B])
                nc.vector.tensor_copy(zbagT8[kk][:, j, 0:B], ptr)

        # ---- S1 own rows (raw lhsT); stash raw psum in SBUF ----
        s1rs = persist.tile([128, TPC], F32, name="s1rs")
        pos = persist.tile([128, TPC], F32, name="pos")
        es1 = persist.tile([128, TPC, B], F32, name="es1")
        s1sc = persist.tile([128, B], F32, name="s1sc")
        s1raw = persist.tile([128, TPC, B], F32, name="s1raw")
        for t in range(TPC):
            pm = ps_sm.tile([128, 16], F32, tag="sm", name="ps_s1")
            for kk in range(2):
                nc.tensor.matmul(
                    pm,
                    lhsT=yt8[kk][:, :, t * 128:(t + 1) * 128],
                    rhs=zbagT8[kk],
                    start=(kk == 0), stop=(kk == 1),
                    perf_mode=DR,
                )
            nc.vector.tensor_copy(s1raw[:, t, :], pm[:, 0:B])

        # ---- sumsq via squares + PE ones-matmul partition reduce ----
        sq16 = [persist.tile([128, 2, BS], BF16, name=f"sq16_{kk}") for kk in range(2)]
        ssd = [dram.tile([1, 1024], F32, name=f"ssd_{j}") for j in range(NPAIR)]
        ssrow = [persist.tile([8, 1024], F32, name=f"ssrow_{j}") for j in range(NPAIR)]
        ss = persist.tile([128, NBLK * TPC], F32, name="ss")
        rb = persist.tile([128, NBLK * TPC], F32, name="rb")
        rbc = [persist.tile([128, 1024], BF16, name=f"rbc_{j}") for j in range(NPAIR)]
        rbd = [dram.tile([1, 1024], BF16, name=f"rbd_{j}") for j in range(NPAIR)]

        def sumsq_quarter(cq):
            """squares (spread across engines) + ones-matmul partition
            reduce into ssrow[cq], bounced to DRAM"""
            c0 = cq * 1024
            for kk in range(2):
                for j in range(2):
                    if cq == 0:
                        eng = nc.scalar if kk == 0 else nc.vector
                    elif cq == 1:
                        eng = nc.scalar
                    elif cq == 2:
                        eng = nc.vector
                    else:
                        eng = nc.scalar if kk == 0 else nc.gpsimd
                    if eng is nc.scalar:
                        eng.activation(
                            sq16[kk][:, j, c0:c0 + 1024],
                            yt8[kk][:, j, c0:c0 + 1024],
                            mybir.ActivationFunctionType.Square,
                        )
                    else:
                        eng.tensor_mul(
                            sq16[kk][:, j, c0:c0 + 1024],
                            yt8[kk][:, j, c0:c0 + 1024],
                            yt8[kk][:, j, c0:c0 + 1024],
                        )
            for h in range(2):
                cb = cq * 2 + h
                pss = ps_sm.tile([8, 512], F32, tag="sm", name=f"ps_ss{cb}")
                first = True
                for kk in range(2):
                    for j in range(2):
                        nc.tensor.matmul(
                            pss, lhsT=ones8,
                            rhs=sq16[kk][:, j, cb * 512:(cb + 1) * 512],
                            start=first, stop=(kk == 1 and j == 1),
                        )
                        first = False
                if cq < 2:
                    nc.vector.tensor_copy(ssrow[cq][:, h * 512:(h + 1) * 512], pss)
                else:
                    # keep the vector queue clear for the pair-0/1 chains:
                    # these copies would otherwise block them behind the
                    # slower gpsimd squares (in-order queue)
                    nc.scalar.copy(ssrow[cq][:, h * 512:(h + 1) * 512], pss)
            if cq < 2:
                nc.sync.dma_start(out=ssd[cq], in_=ssrow[cq][0:1, :])
            else:
                # sync is in-order: these wait on the scalar copies above and
                # would block the pair-0/1 ssback DMAs behind them
                nc.scalar.dma_start(out=ssd[cq], in_=ssrow[cq][0:1, :])

        def rsqrt_pair(pj, prescale_scale):
            """rb for pair pj from ssd[pj]; broadcast tile rbc[pj]
            (times 16 for prescale pairs: zt8 = y16*16*rb = 16*z)."""
            c0 = pj * 8
            nc.sync.dma_start(
                out=ss[:, c0:c0 + 8],
                in_=ssd[pj].rearrange("1 (m p) -> p m", p=128),
            )
            nc.vector.tensor_scalar(
                out=rb[:, c0:c0 + 8], in0=ss[:, c0:c0 + 8],
                scalar1=-LIN_B, scalar2=LIN_A,
                op0=mybir.AluOpType.mult, op1=mybir.AluOpType.add,
            )
            _newton(nc, work, rb[:, c0:c0 + 8], ss[:, c0:c0 + 8], 8, f"p{pj}")
            ptr = ps_sm.tile([8, 128], F32, tag="sm", name=f"ptr_rb{pj}")
            nc.tensor.transpose(ptr, rb[:, c0:c0 + 8], ident)
            rT = work.tile([8, 128], BF16, name=f"rT_{pj}")
            nc.vector.tensor_scalar(
                out=rT, in0=ptr, scalar1=prescale_scale, scalar2=None,
                op0=mybir.AluOpType.mult,
            )
            nc.sync.dma_start(
                out=rbd[pj].rearrange("1 (t p) -> t p", t=8), in_=rT
            )
            # 4 parallel 64KB bf16 broadcast DMAs (a single 512KB f32
            # broadcast runs ~35us on one queue; K=1 PE matmuls steal
            # ~0.7us each from the main-loop stream)
            for q in range(4):
                nc.sync.dma_start(
                    out=rbc[pj][q * 32:(q + 1) * 32, :],
                    in_=rbd[pj].to_broadcast((32, 1024)),
                )

        sumsq_quarter(0)
        sumsq_quarter(1)
        sumsq_quarter(2)
        sumsq_quarter(3)
        rsqrt_pair(0, 1.0)
        rsqrt_pair(1, 1.0)
        rsqrt_pair(2, 1.0)
        rsqrt_pair(3, 1.0)

        # scale APs: r2own = 2*rb_own (raw+fixup), rs1own = rb_own/8 (prescaled/S1)
        r2own = persist.tile([128, TPC], F32, name="r2own")
        nc.vector.tensor_scalar(
            out=r2own, in0=rb[:, 0:TPC], scalar1=2.0, scalar2=None,
            op0=mybir.AluOpType.mult,
        )
        rs1own = persist.tile([128, TPC], F32, name="rs1own")
        nc.vector.tensor_scalar(
            out=rs1own, in0=rb[:, 0:TPC], scalar1=0.125, scalar2=None,
            op0=mybir.AluOpType.mult,
        )

        # es1 + positives (s1raw = 256 * y.zbag)
        for t in range(TPC):
            nc.scalar.activation(
                es1[:, t, :], s1raw[:, t, :], mybir.ActivationFunctionType.Exp,
                scale=rs1own[:, t:t + 1], accum_out=s1rs[:, t:t + 1],
            )
            nc.vector.tensor_mul(s1sc, s1raw[:, t, :], oh)
            nc.vector.reduce_sum(pos[:, t:t + 1], s1sc, axis=mybir.AxisListType.X)
        nc.vector.tensor_mul(pos, pos, rb[:, 0:TPC])  # pos = 16*sim, folded later

        # ---- Bgram ----
        pbg = ps_sm.tile([B, B], F32, tag="sm", name="ps_bgram")
        for kk in range(2):
            for j in range(2):
                nc.tensor.matmul(
                    pbg, lhsT=zbagT8[kk][:, j, 0:B], rhs=zbagT8[kk][:, j, 0:B],
                    start=(kk == 0 and j == 0), stop=(kk == 1 and j == 1),
                )
        ebg = persist.tile([B, B], F32, name="exp_bgram")
        rsbg = persist.tile([B, 1], F32, name="rs_bgram")
        nc.scalar.activation(
            ebg, pbg, mybir.ActivationFunctionType.Exp, scale=2.0 / 256.0,
            accum_out=rsbg,
        )

        # colsum over own rows of exp(2 S1own) + rsbg row: off the tail
        pv = ps_sm.tile([1, B], F32, tag="sm", name="ps_v")
        for t in range(TPC):
            nc.tensor.matmul(
                pv, lhsT=ones, rhs=es1[:, t, :],
                start=(t == 0), stop=(t == TPC - 1),
            )
        vrow = persist.tile([1, B], F32, name="vrow")
        nc.vector.tensor_copy(vrow, pv)
        prb = ps_sm.tile([1, B], F32, tag="sm", name="ps_rbT")
        nc.tensor.transpose(prb, rsbg, ident[:B, :B])
        rsbgT = persist.tile([1, B], F32, name="rsbgT")
        nc.vector.tensor_copy(rsbgT, prb)

        # ---- main loop ----
        rs = persist.tile([128, TPC, NPAIR], F32, name="rs")
        for bb in range(NPAIR):
            for t in range(TPC):
                pm = ps_main.tile([128, 1024], F32, name="ps_g")
                for kk in range(2):
                    for half in range(2):
                        blk = 2 * bb + half
                        nc.tensor.matmul(
                            pm[:, half * 512:(half + 1) * 512],
                            lhsT=yt8[kk][:, :, t * 128:(t + 1) * 128],
                            rhs=yt8[kk][:, :, blk * 512:(blk + 1) * 512],
                            start=(kk == 0), stop=(kk == 1),
                            perf_mode=DR,
                        )
                # fixup writes bf16 to SBUF: the PSUM tile frees here (not
                # after the exp), shortening the PE's psum-recycle chain
                fx = expool.tile([128, 1024], BF16, name="fx")
                nc.vector.tensor_mul(fx, pm, rbc[bb])
                nc.scalar.activation(
                    fx, fx, mybir.ActivationFunctionType.Exp,
                    scale=r2own[:, t:t + 1], accum_out=rs[:, t, bb:bb + 1],
                )

        # ---- denominators + logs ----
        rsum = persist.tile([128, TPC], F32, name="rsum")
        nc.vector.reduce_sum(rsum, rs, axis=mybir.AxisListType.X)
        di = persist.tile([128, TPC], F32, name="di")
        nc.vector.tensor_scalar(
            out=di, in0=s1rs, scalar1=512.0, scalar2=-E2,
            op0=mybir.AluOpType.mult, op1=mybir.AluOpType.add,
        )
        nc.vector.tensor_add(di, di, rsum)
        ldi = persist.tile([128, TPC], F32, name="ldi")
        nc.scalar.activation(ldi, di, mybir.ActivationFunctionType.Ln)

        # fin = sum_t ldi - (4/16)*sum_t pos
        fin = persist.tile([128, 1], F32, name="fin")
        vsum = persist.tile([128, 1], F32, name="vsum")
        nc.vector.reduce_sum(vsum, ldi, axis=mybir.AxisListType.X)
        posr = persist.tile([128, 1], F32, name="posr")
        nc.vector.reduce_sum(posr, pos, axis=mybir.AxisListType.X)
        nc.vector.tensor_scalar(
            out=posr, in0=posr, scalar1=-0.25, scalar2=None,
            op0=mybir.AluOpType.mult,
        )
        nc.vector.tensor_add(fin, vsum, posr)

        pfin = ps_sm.tile([1, 1], F32, tag="sm", name="ps_fin")
        nc.tensor.matmul(pfin, lhsT=ones, rhs=fin, start=True, stop=True)
        outt = persist.tile([1, 18], F32, name="outt")
        nc.vector.memset(outt, 0.0)
        nc.vector.tensor_copy(outt[:, 0:1], pfin)
        nc.vector.tensor_copy(outt[:, 2:10], vrow)
        nc.vector.tensor_copy(outt[:, 10:18], rsbgT)
        nc.sync.dma_start(out=out_d[:, :], in_=outt)

    return nc


def _get_nc():
    global _CACHED_NC
    if _CACHED_NC is None:
        nc = bacc.Bacc(
            "TRN2", target_bir_lowering=False, debug=False, num_devices=NCORES
        )
        nc = _build_kernel(nc)
        nc.compile()
        _CACHED_NC = nc
    return _CACHED_NC


def kernel(instance_emb: np.ndarray, bag_emb: np.ndarray) -> np.ndarray:
    global LAST_EXEC_TIME_NS
    Y = np.asarray(instance_emb, dtype=np.float32).reshape(BS, D)
    bg = np.ascontiguousarray(np.asarray(bag_emb, dtype=np.float32))

    in_maps = []
    for c in range(NCORES):
        Yc = np.roll(Y, -c * RPC, axis=0)
        yt16 = np.clip(Yc.T * 16.0, -240.0, 240.0)
        ytb_c = np.ascontiguousarray(yt16.astype(ml_dtypes.float8_e4m3))
        ohv = np.zeros((1, B), np.float32)
        ohv[0, c] = 1.0
        in_maps.append({"ytb": ytb_c, "bag": bg, "onehot": ohv})

    nc = _get_nc()
    trace = os.environ.get("CL_KERNEL_TRACE", "0") == "1"
    tmpdir = os.environ.get("CL_KERNEL_TRACE_DIR") or None
    if os.environ.get("CL_KERNEL_WARMUP", "0") == "1":
        bass_utils.run_bass_kernel_spmd(
            nc, in_maps, core_ids=list(range(NCORES)), trace=False
        )
    res = bass_utils.run_bass_kernel_spmd(
        nc, in_maps, core_ids=list(range(NCORES)), trace=trace, tmpdir=tmpdir
    )
    LAST_EXEC_TIME_NS = res.exec_time_ns

    return _assemble([res.results[c]["out"] for c in range(NCORES)])


def _assemble(outs) -> np.ndarray:
    """out row = [partial_c, pad, v_c[0:8], rsbg[0:8]];
    denom_j[g] = sum_c v_c[g] + 512*rsbg[g] - e^2."""
    total = 0.0
    vsum = np.zeros(B, np.float64)
    for c in range(NCORES):
        o = np.asarray(outs[c], np.float64).reshape(-1)
        total += o[0]
        vsum += o[2:10]
    rsbg = np.asarray(outs[0], np.float64).reshape(-1)[10:18]
    denom_j = vsum + 512.0 * rsbg - E2
    lj = 512.0 * float(np.sum(np.log(denom_j)))
    return np.float32((total + lj) / (2 * BS))
